# revision 2
# baseline (speedup 1.0000x reference)
"""Trainium2 Bass kernel for nn_ContactMapDistError — v3 (DVE+ACT reduce).

Computes, for each batch element b:
    mean over active contact pairs (r,s) of
      min_{v in region r, w in region s} || g1[b,r,v] - g2[b,s,w] ||

Strategy
--------
Host (cheap, O(B*R*VR)):
  - gather region vertex subsets g1, g2 via rid_to_vid
  - build K=5 feature matrices so one matmul yields pairwise squared
    distances: d2(v,w) = [-2x,-2y,-2z,sq1,1]_v . [x',y',z',1,sq2]_w
  - finish the v-axis min (segmented, tiny), sqrt, contact-mask mean

Device (8 cores SPMD; core i -> batch i//2, r-half i%2):
  - PE: fp32r matmuls fill [128, 1536] PSUM tiles (3 banks each, 2-slot
    parity ring), one 512-col matmul per bank.
  - The w-axis min (96 -> 1 per s-region) is the bottleneck: DVE
    tensor_reduce alone is ~93us busy vs PE ~35us. Hardware limits the
    options: GPSIMD has no PSUM port and no generic elementwise ops in
    this toolchain; tensor_tensor cannot read two PSUM operands; ScalarE
    has no min. So tiles are split between two paths:
      D: DVE grouped tensor_reduce straight from PSUM      (DVE 1725ns)
      Z: ACT copies the tile PSUM->SBUF bf16 in 2 bank-aligned pieces
         (chasing PE, early per-bank PSUM frees), then DVE runs a bf16
         2x_1p tensor_tensor min-tree + final reduce; trees of adjacent
         Z-tile pairs are fused to amortize per-inst overheads
                                     (ACT ~1566ns, DVE ~1020-1140ns)
    The ~13/41 mix balances DVE and ACT at ~65us each.
  - every instruction carries at most ONE semaphore update (ISA limit)
  - SP: split input DMA + 3 overlapped output drains (HWDGE)
"""

import sys

sys.path.insert(0, "/opt/trn_rl_repo")

import numpy as np

import concourse.bass as bass
import concourse.mybir as mybir
from concourse.bass_utils import run_bass_kernel_spmd

F32 = mybir.dt.float32
F32R = mybir.dt.float32r
BF16 = mybir.dt.bfloat16
MIN = mybir.AluOpType.min
AXX = mybir.AxisListType.X

B, N, R, VR = 4, 10475, 48, 96
NCORES = 8
RH = R // 2            # r-regions handled per core
V = RH * VR            # packed v columns per core = 2304
T = V // 128           # v-chunks of 128 partitions = 18
W = R * VR             # full w width = 4608
WC = 1536              # psum tile width (3 banks, 16 s-regions)
NWC = W // WC          # = 3
K = 5                  # contraction dim
NK = T * NWC           # total tile count = 54
G = WC // VR           # s-groups per tile = 16

# ---- tunables -------------------------------------------------------------
ND = 17                       # tiles on the D path (rest are Z)
DELTA_Z = 2                   # DVE-queue lag for Z-path trees
CHASE = True                  # ACT pieceA chases PE's m1 (else waits full)
ONEPIECE = False              # single full-tile ACT copy (less ACT busy,
                              # worse PSUM-ring overlap)
NEB = 6                       # ebuf ring (Z-path bf16 copies)
PAIR = True                   # fuse trees of adjacent Z-tile pairs
DRAIN_RANGES = [(0, 30), (30, 48), (48, 54)]   # tile ranges per output DMA

_cache = {}


def _mk_paths():
    """Spread D tiles evenly among the Z tiles (the 2-slot PSUM ring
    prefers alternating DVE- and ACT-consumed tiles). Tile 0 is forced
    to D so DVE's first reduce only waits for the first PSUM fill."""
    slots = ["D" if i * ND // NK > (i - 1) * ND // NK else "Z"
             for i in range(NK)]
    return slots


def _grp(ap, w):
    return ap.rearrange("p (g w) -> p g w", w=w)


def _build(L=1):
    if ("nc", L) in _cache:
        return _cache[("nc", L)]
    paths = _mk_paths()
    nc = bass.Bass()
    ab = nc.declare_dram_parameter("ab", [K, V + W], F32R, isOutput=False)
    s1out = nc.declare_dram_parameter("s1out", [128, NK * G], F32, isOutput=True)

    abt = nc.alloc_sbuf_tensor("abt", [K, V + W], F32R).ap()
    s1buf = nc.alloc_sbuf_tensor("s1buf", [128, NK * G], F32).ap()
    pts = [nc.alloc_psum_tensor(f"pt{i}", [128, WC], F32).ap() for i in range(2)]

    # Z-path scratch (bf16). ebuf slots are allocated as one tensor so a
    # fused pair-tree can address two consecutive slots with one AP.
    ebuf_all = nc.alloc_sbuf_tensor("eb", [128, NEB * WC], BF16).ap()
    ebuf = [ebuf_all[:, i * WC : (i + 1) * WC] for i in range(NEB)]
    e48 = [nc.alloc_sbuf_tensor(f"e48_{i}", [128, 2 * G * 48], BF16).ap() for i in range(2)]
    e24 = [nc.alloc_sbuf_tensor(f"e24_{i}", [128, 2 * G * 24], BF16).ap() for i in range(2)]
    e12 = [nc.alloc_sbuf_tensor(f"e12_{i}", [128, 2 * G * 12], BF16).ap() for i in range(2)]

    lt = abt[:, 0:V]
    rt = abt[:, V : V + W]

    # ---- static schedule bookkeeping ----
    # tile order is c-outer (k = c*T + t) so the first input DMA pieces
    # (lhsT + first rhs chunk) unblock the whole first third.
    meta = []
    iz = 0
    actpar = [0, 0]
    dpar = [0, 0]
    for k in range(NK):
        p = paths[k]
        c, t = divmod(k, T)
        par = k % 2
        m = {"path": p, "k": k, "c": c, "t": t, "par": par}
        if p == "D":
            m["idp"] = dpar[par]
            dpar[par] += 1
        else:
            m["iap"] = actpar[par]
            actpar[par] += 1
            m["iz"] = iz
            iz += 1
        meta.append(m)
    NZ = iz
    NDPAR = list(dpar)
    NAPAR = list(actpar)
    zs = [m for m in meta if m["path"] == "Z"]

    # pair up Z tiles whose ebuf slots are adjacent (even iz with its
    # successor) for fused trees; leftover tiles get a solo tree.
    groups = []           # list of [m] or [m1, m2]
    if PAIR:
        i = 0
        while i < NZ:
            if i + 1 < NZ:
                groups.append([zs[i], zs[i + 1]])
                i += 2
            else:
                groups.append([zs[i]])
                i += 1
    else:
        groups = [[m] for m in zs]
    for gi, grp_ms in enumerate(groups):
        for m in grp_ms:
            m["grp"] = gi

    # DVE queue emit position of each tile's final s1buf write
    def emit_pos(m):
        if m["path"] == "Z":
            g = groups[m["grp"]]
            return (g[-1]["k"] + DELTA_Z, 1)
        return (m["k"], 0)

    fin_marker = {}
    for third, (k0, k1) in enumerate(DRAIN_RANGES):
        last = max((m for m in meta if k0 <= m["k"] < k1), key=emit_pos)
        assert last["path"] == "Z", (
            "drain-range boundary must land on a Z tile so the fin inc can "
            "ride its final reduce")
        # the inc rides the group's final reduce; key by the group's last k
        fin_marker[groups[last["grp"]][-1]["k"]] = third

    with (
        nc.Block() as block,
        nc.semaphore("dma_in") as dma_in,
        nc.semaphore("dma_in2") as dma_in2,
        nc.semaphore("dma_in3") as dma_in3,
        nc.semaphore("pe_sem") as pe_sem,
        nc.semaphore("peb1") as peb1,
        nc.semaphore("dveF0") as dveF0,
        nc.semaphore("dveF1") as dveF1,
        nc.semaphore("actA0") as actA0,
        nc.semaphore("actA1") as actA1,
        nc.semaphore("actB0") as actB0,
        nc.semaphore("actB1") as actB1,
        nc.semaphore("a2rd") as a2rd,
        nc.semaphore("fin_sem") as fin_sem,
        nc.semaphore("out_sem") as out_sem,
    ):
        dveF = [dveF0, dveF1]
        actA = [actA0, actA1]
        actB = [actB0, actB1]

        @block.sync
        def _(sp):
            for j in range(L):
                if j > 0:
                    sp.wait_ge(pe_sem, NK * j)
                # piece 1: all of lhsT + first bank of rhs chunk 0 (m0 of
                # tile 0 can start); piece 2: rest of rhs chunk 0; piece 3:
                # rest. Separate semaphores: DMA completions are unordered.
                sp.dma_start(abt[:, 0 : V + 512],
                             ab[:, 0 : V + 512]).then_inc(dma_in, 16)
                sp.dma_start(abt[:, V + 512 : V + WC],
                             ab[:, V + 512 : V + WC]).then_inc(dma_in3, 16)
                sp.dma_start(abt[:, V + WC :], ab[:, V + WC :]).then_inc(dma_in2, 16)
                for third, (k0, k1) in enumerate(DRAIN_RANGES):
                    cols = slice(k0 * G, k1 * G)
                    sp.wait_ge(fin_sem, 3 * j + third + 1)
                    sp.dma_start(s1out[:, cols], s1buf[:, cols]).then_inc(out_sem, 16)
            sp.wait_ge(out_sem, 48 * L)

        @block.tensor
        def _(pe):
            for j in range(L):
                for m in meta:
                    k, par, c, t = m["k"], m["par"], m["c"], m["t"]
                    if k == 0:
                        pe.wait_ge(dma_in, 16 * (j + 1))
                    elif k == T:
                        pe.wait_ge(dma_in2, 16 * (j + 1))
                    pt = pts[par]
                    # refill gating: wait on the consumer of tile k-2
                    # (same parity slot); at iteration start, on the last
                    # same-parity tile of the previous iteration
                    if k >= 2:
                        pv, joff = meta[k - 2], 0
                    elif j > 0:
                        pv, joff = meta[NK - 2 + k], -1
                    else:
                        pv = None
                    for mm in range(3):
                        if k == 0 and mm == 1:
                            pe.wait_ge(dma_in3, 16 * (j + 1))
                        if pv is not None:
                            if pv["path"] == "D":
                                if mm == 0:
                                    pe.wait_ge(
                                        dveF[par],
                                        NDPAR[par] * (j + joff) + pv["idp"] + 1)
                            else:
                                if mm == 0:
                                    pe.wait_ge(
                                        actB[par] if ONEPIECE else actA[par],
                                        NAPAR[par] * (j + joff) + pv["iap"] + 1)
                                elif mm == 2 and not ONEPIECE:
                                    pe.wait_ge(
                                        actB[par],
                                        NAPAR[par] * (j + joff) + pv["iap"] + 1)
                        inst = pe.matmul(
                            pt[:, mm * 512 : (mm + 1) * 512],
                            lt[:, t * 128 : (t + 1) * 128],
                            rt[:, c * WC + mm * 512 : c * WC + (mm + 1) * 512],
                            start=True, stop=True)
                        if mm == 1:
                            inst.then_inc(peb1)
                        elif mm == 2:
                            inst.then_inc(pe_sem)

        @block.scalar
        def _(s):
            for j in range(L):
                for m in meta:
                    if m["path"] == "D":
                        continue
                    k, par, i = m["k"], m["par"], m["iz"]
                    if i >= NEB or j > 0:
                        # ebuf slot free once the tree NEB back read it
                        base = NZ * j + (-NZ if i < NEB else 0)
                        s.wait_ge(a2rd, base + (i - NEB) + 1)
                    dst = ebuf[i % NEB]
                    if ONEPIECE:
                        s.wait_ge(pe_sem, NK * j + k + 1)
                        s.copy(dst, pts[par]).then_inc(actB[par])
                    else:
                        # chase PE: banks 0-1 after its m1, then bank 2
                        if CHASE:
                            s.wait_ge(peb1, NK * j + k + 1)
                        else:
                            s.wait_ge(pe_sem, NK * j + k + 1)
                        s.copy(dst[:, 0:1024], pts[par][:, 0:1024]).then_inc(actA[par])
                        s.wait_ge(pe_sem, NK * j + k + 1)
                        s.copy(dst[:, 1024:WC], pts[par][:, 1024:WC]).then_inc(actB[par])

        @block.vector
        def _(v):
            for j in range(L):
                queue = []

                def emit_D(m, j=j):
                    k, par = m["k"], m["par"]

                    def go():
                        v.wait_ge(pe_sem, NK * j + k + 1)
                        v.tensor_reduce(
                            s1buf[:, k * G : (k + 1) * G], _grp(pts[par], 96),
                            axis=AXX, op=MIN).then_inc(dveF[par])
                    return go

                def emit_Ztree(ms, j=j):
                    # fused tree over 1 or 2 Z tiles; for 2, their ebuf
                    # slots are consecutive (iz even,odd with same %NEB
                    # pairing guaranteed by NEB even)
                    n = len(ms)
                    last = ms[-1]
                    i0 = ms[0]["iz"] % NEB
                    src = ebuf_all[:, i0 * WC : (i0 + n) * WC]
                    b48 = e48[(ms[0]["iz"] // 2) % 2][:, 0 : n * G * 48]
                    b24 = e24[(ms[0]["iz"] // 2) % 2][:, 0 : n * G * 24]
                    b12 = e12[(ms[0]["iz"] // 2) % 2][:, 0 : n * G * 12]
                    # output columns: per-tile 16-col blocks (uniform
                    # stride only if adjacent; handle via separate reduces)

                    def go():
                        v.wait_ge(actB[last["par"]],
                                  NAPAR[last["par"]] * j + last["iap"] + 1)
                        sg = _grp(src, 96)
                        v.tensor_tensor(
                            _grp(b48, 48), sg[:, :, 0:48], sg[:, :, 48:96],
                            op=MIN).then_inc(a2rd)
                        t2 = v.tensor_tensor(
                            _grp(b24, 24), _grp(b48, 48)[:, :, 0:24],
                            _grp(b48, 48)[:, :, 24:48], op=MIN)
                        if n == 2:
                            # second ebuf slot is also done after tt48, but
                            # the free must ride on a compute inst so it
                            # fires after the engine drains
                            t2.then_inc(a2rd)
                        v.tensor_tensor(
                            _grp(b12, 12), _grp(b24, 24)[:, :, 0:12],
                            _grp(b24, 24)[:, :, 12:24], op=MIN)
                        for idx, m in enumerate(ms):
                            k = m["k"]
                            r = v.tensor_reduce(
                                s1buf[:, k * G : (k + 1) * G],
                                _grp(b12[:, idx * G * 12 : (idx + 1) * G * 12], 12),
                                axis=AXX, op=MIN)
                        if last["k"] in fin_marker:
                            r.then_inc(fin_sem)
                    return go

                for m in meta:
                    if m["path"] == "D":
                        queue.append((emit_pos(m) + (0,), emit_D(m)))
                for g in groups:
                    queue.append((emit_pos(g[0]) + (0,), emit_Ztree(g)))
                queue.sort(key=lambda q: q[0])
                if j > 0:
                    v.wait_ge(out_sem, 48 * j)
                for _, go in queue:
                    go()

    _cache[("nc", L)] = nc
    return nc


def _prep_inputs(v1s, v2s, rid_to_vid):
    """Build per-core fused lhsT|rhs feature matrices."""
    g1 = v1s[:, rid_to_vid, :]  # [B, R, VR, 3]
    g2 = v2s[:, rid_to_vid, :]
    g1_64 = g1.astype(np.float64)
    g2_64 = g2.astype(np.float64)
    sq1 = (g1_64 * g1_64).sum(-1)  # [B, R, VR]
    sq2 = (g2_64 * g2_64).sum(-1)

    in_maps = []
    for core in range(NCORES):
        b, h = divmod(core, 2)
        rs = slice(RH * h, RH * (h + 1))
        a = np.empty((K, V + W), np.float32)
        a[0:3, 0:V] = -2.0 * g1[b, rs].reshape(V, 3).T
        a[3, 0:V] = sq1[b, rs].reshape(V).astype(np.float32)
        a[4, 0:V] = 1.0
        a[0:3, V:] = g2[b].reshape(W, 3).T
        a[3, V:] = 1.0
        a[4, V:] = sq2[b].reshape(W).astype(np.float32)
        in_maps.append({"ab": a})
    return in_maps


def kernel(v1s, v2s, cmaps, rid_to_vid):
    v1s = np.asarray(v1s)
    v2s = np.asarray(v2s)
    cmaps = np.asarray(cmaps)
    rid_to_vid = np.asarray(rid_to_vid)

    nc = _build()
    in_maps = _prep_inputs(v1s, v2s, rid_to_vid)
    res = run_bass_kernel_spmd(nc, in_maps, core_ids=list(range(NCORES)))

    # assemble [B, R, R] min squared distances (r = person1 region rows)
    md2 = np.empty((B, R, R), np.float32)
    for core in range(NCORES):
        b, h = divmod(core, 2)
        out = res.results[core]["s1out"]  # [128, NK*G], k = c*T + t
        # (p, c, t, g) -> v = t*128 + p, s = c*G + g
        per_v = out.reshape(128, NWC, T, G).transpose(2, 0, 1, 3).reshape(V, R)
        # segmented min over each region's 96 rows
        md2[b, RH * h : RH * (h + 1), :] = per_v.reshape(RH, VR, R).min(axis=1)

    md = np.sqrt(np.maximum(md2, 0.0))
    m = cmaps.astype(np.float32)
    return ((md * m).sum(axis=(1, 2)) / m.sum(axis=(1, 2))).astype(np.float32)


# revision 3
# speedup vs baseline: 1.0149x; 1.0149x over previous
"""Trainium2 Bass kernel for nn_ContactMapDistError — v3 (DVE+ACT reduce).

Computes, for each batch element b:
    mean over active contact pairs (r,s) of
      min_{v in region r, w in region s} || g1[b,r,v] - g2[b,s,w] ||

Strategy
--------
Host (cheap, O(B*R*VR)):
  - gather region vertex subsets g1, g2 via rid_to_vid
  - build K=5 feature matrices so one matmul yields pairwise squared
    distances: d2(v,w) = [-2x,-2y,-2z,sq1,1]_v . [x',y',z',1,sq2]_w
  - finish the v-axis min (segmented, tiny), sqrt, contact-mask mean

Device (8 cores SPMD; core i -> batch i//2, r-half i%2):
  - PE: fp32r matmuls fill [128, 1536] PSUM tiles (3 banks each, 2-slot
    parity ring), one 512-col matmul per bank.
  - The w-axis min (96 -> 1 per s-region) is the bottleneck: DVE
    tensor_reduce alone is ~93us busy vs PE ~35us. Hardware limits the
    options: GPSIMD has no PSUM port and no generic elementwise ops in
    this toolchain; tensor_tensor cannot read two PSUM operands; ScalarE
    has no min. So tiles are split between two paths:
      D: DVE grouped tensor_reduce straight from PSUM      (DVE 1725ns)
      Z: ACT copies the tile PSUM->SBUF bf16 in 2 bank-aligned pieces
         (chasing PE, early per-bank PSUM frees), then DVE runs a bf16
         2x_1p tensor_tensor min-tree + final reduce; trees of adjacent
         Z-tile pairs are fused to amortize per-inst overheads
                                     (ACT ~1566ns, DVE ~1020-1140ns)
    The 18/36 mix balances DVE (~66us busy) and ACT (~60us busy).
  - every instruction carries at most ONE semaphore update (ISA limit)
  - SP: split input DMA + 3 overlapped output drains (HWDGE)
"""

import sys

sys.path.insert(0, "/opt/trn_rl_repo")

import numpy as np

import concourse.bass as bass
import concourse.mybir as mybir
from concourse.bass_utils import run_bass_kernel_spmd

F32 = mybir.dt.float32
F32R = mybir.dt.float32r
BF16 = mybir.dt.bfloat16
MIN = mybir.AluOpType.min
AXX = mybir.AxisListType.X

B, N, R, VR = 4, 10475, 48, 96
NCORES = 8
RH = R // 2            # r-regions handled per core
V = RH * VR            # packed v columns per core = 2304
T = V // 128           # v-chunks of 128 partitions = 18
W = R * VR             # full w width = 4608
WC = 1536              # psum tile width (3 banks, 16 s-regions)
NWC = W // WC          # = 3
K = 5                  # contraction dim
NK = T * NWC           # total tile count = 54
G = WC // VR           # s-groups per tile = 16

# ---- tunables -------------------------------------------------------------
ND = 18                       # tiles on the D path (rest are Z)
DELTA_Z = 2                   # DVE-queue lag for Z-path trees
CHASE = True                  # ACT pieceA chases PE's m1 (else waits full)
ONEPIECE = False              # single full-tile ACT copy (less ACT busy,
                              # worse PSUM-ring overlap)
NEB = 6                       # ebuf ring (Z-path bf16 copies)
PAIR = True                   # fuse trees of adjacent Z-tile pairs
DRAIN_RANGES = [(0, 36), (36, 50), (50, 54)]   # tile ranges per output DMA

_cache = {}


def _mk_paths():
    """Spread D tiles evenly among the Z tiles (the 2-slot PSUM ring
    prefers alternating DVE- and ACT-consumed tiles). Tile 0 is forced
    to D so DVE's first reduce only waits for the first PSUM fill."""
    slots = ["D" if i * ND // NK > (i - 1) * ND // NK else "Z"
             for i in range(NK)]
    return slots


def _grp(ap, w):
    return ap.rearrange("p (g w) -> p g w", w=w)


def _build(L=1):
    if ("nc", L) in _cache:
        return _cache[("nc", L)]
    paths = _mk_paths()
    nc = bass.Bass()
    ab = nc.declare_dram_parameter("ab", [K, V + W], F32R, isOutput=False)
    s1out = nc.declare_dram_parameter("s1out", [128, NK * G], F32, isOutput=True)

    abt = nc.alloc_sbuf_tensor("abt", [K, V + W], F32R).ap()
    s1buf = nc.alloc_sbuf_tensor("s1buf", [128, NK * G], F32).ap()
    pts = [nc.alloc_psum_tensor(f"pt{i}", [128, WC], F32).ap() for i in range(2)]

    # Z-path scratch (bf16). ebuf slots are allocated as one tensor so a
    # fused pair-tree can address two consecutive slots with one AP.
    ebuf_all = nc.alloc_sbuf_tensor("eb", [128, NEB * WC], BF16).ap()
    ebuf = [ebuf_all[:, i * WC : (i + 1) * WC] for i in range(NEB)]
    e48 = [nc.alloc_sbuf_tensor(f"e48_{i}", [128, 2 * G * 48], BF16).ap() for i in range(2)]
    e24 = [nc.alloc_sbuf_tensor(f"e24_{i}", [128, 2 * G * 24], BF16).ap() for i in range(2)]
    e12 = [nc.alloc_sbuf_tensor(f"e12_{i}", [128, 2 * G * 12], BF16).ap() for i in range(2)]

    lt = abt[:, 0:V]
    rt = abt[:, V : V + W]

    # ---- static schedule bookkeeping ----
    # tile order is c-outer (k = c*T + t) so the first input DMA pieces
    # (lhsT + first rhs chunk) unblock the whole first third.
    meta = []
    iz = 0
    actpar = [0, 0]
    dpar = [0, 0]
    for k in range(NK):
        p = paths[k]
        c, t = divmod(k, T)
        par = k % 2
        m = {"path": p, "k": k, "c": c, "t": t, "par": par}
        if p == "D":
            m["idp"] = dpar[par]
            dpar[par] += 1
        else:
            m["iap"] = actpar[par]
            actpar[par] += 1
            m["iz"] = iz
            iz += 1
        meta.append(m)
    NZ = iz
    NDPAR = list(dpar)
    NAPAR = list(actpar)
    zs = [m for m in meta if m["path"] == "Z"]

    # pair up Z tiles whose ebuf slots are adjacent (even iz with its
    # successor) for fused trees; leftover tiles get a solo tree.
    groups = []           # list of [m] or [m1, m2]
    if PAIR:
        i = 0
        while i < NZ:
            if i + 1 < NZ:
                groups.append([zs[i], zs[i + 1]])
                i += 2
            else:
                groups.append([zs[i]])
                i += 1
    else:
        groups = [[m] for m in zs]
    for gi, grp_ms in enumerate(groups):
        for m in grp_ms:
            m["grp"] = gi

    # DVE queue emit position of each tile's final s1buf write
    def emit_pos(m):
        if m["path"] == "Z":
            g = groups[m["grp"]]
            return (g[-1]["k"] + DELTA_Z, 1)
        return (m["k"], 0)

    fin_marker = {}
    for third, (k0, k1) in enumerate(DRAIN_RANGES):
        last = max((m for m in meta if k0 <= m["k"] < k1), key=emit_pos)
        assert last["path"] == "Z", (
            "drain-range boundary must land on a Z tile so the fin inc can "
            "ride its final reduce")
        # the inc rides the group's final reduce; key by the group's last k
        fin_marker[groups[last["grp"]][-1]["k"]] = third

    with (
        nc.Block() as block,
        nc.semaphore("dma_in") as dma_in,
        nc.semaphore("dma_in2") as dma_in2,
        nc.semaphore("dma_in3") as dma_in3,
        nc.semaphore("pe_sem") as pe_sem,
        nc.semaphore("peb1") as peb1,
        nc.semaphore("dveF0") as dveF0,
        nc.semaphore("dveF1") as dveF1,
        nc.semaphore("actA0") as actA0,
        nc.semaphore("actA1") as actA1,
        nc.semaphore("actB0") as actB0,
        nc.semaphore("actB1") as actB1,
        nc.semaphore("a2rd") as a2rd,
        nc.semaphore("fin_sem") as fin_sem,
        nc.semaphore("out_sem") as out_sem,
    ):
        dveF = [dveF0, dveF1]
        actA = [actA0, actA1]
        actB = [actB0, actB1]

        @block.sync
        def _(sp):
            for j in range(L):
                if j > 0:
                    sp.wait_ge(pe_sem, NK * j)
                # piece 1: all of lhsT + first bank of rhs chunk 0 (m0 of
                # tile 0 can start); piece 2: rest of rhs chunk 0; piece 3:
                # rest. Separate semaphores: DMA completions are unordered.
                sp.dma_start(abt[:, 0 : V + 512],
                             ab[:, 0 : V + 512]).then_inc(dma_in, 16)
                sp.dma_start(abt[:, V + 512 : V + WC],
                             ab[:, V + 512 : V + WC]).then_inc(dma_in3, 16)
                sp.dma_start(abt[:, V + WC :], ab[:, V + WC :]).then_inc(dma_in2, 16)
                for third, (k0, k1) in enumerate(DRAIN_RANGES):
                    cols = slice(k0 * G, k1 * G)
                    sp.wait_ge(fin_sem, 3 * j + third + 1)
                    sp.dma_start(s1out[:, cols], s1buf[:, cols]).then_inc(out_sem, 16)
            sp.wait_ge(out_sem, 48 * L)

        @block.tensor
        def _(pe):
            for j in range(L):
                for m in meta:
                    k, par, c, t = m["k"], m["par"], m["c"], m["t"]
                    if k == 0:
                        pe.wait_ge(dma_in, 16 * (j + 1))
                    elif k == T:
                        pe.wait_ge(dma_in2, 16 * (j + 1))
                    pt = pts[par]
                    # refill gating: wait on the consumer of tile k-2
                    # (same parity slot); at iteration start, on the last
                    # same-parity tile of the previous iteration
                    if k >= 2:
                        pv, joff = meta[k - 2], 0
                    elif j > 0:
                        pv, joff = meta[NK - 2 + k], -1
                    else:
                        pv = None
                    for mm in range(3):
                        if k == 0 and mm == 1:
                            pe.wait_ge(dma_in3, 16 * (j + 1))
                        if pv is not None:
                            if pv["path"] == "D":
                                if mm == 0:
                                    pe.wait_ge(
                                        dveF[par],
                                        NDPAR[par] * (j + joff) + pv["idp"] + 1)
                            else:
                                if mm == 0:
                                    pe.wait_ge(
                                        actB[par] if ONEPIECE else actA[par],
                                        NAPAR[par] * (j + joff) + pv["iap"] + 1)
                                elif mm == 2 and not ONEPIECE:
                                    pe.wait_ge(
                                        actB[par],
                                        NAPAR[par] * (j + joff) + pv["iap"] + 1)
                        inst = pe.matmul(
                            pt[:, mm * 512 : (mm + 1) * 512],
                            lt[:, t * 128 : (t + 1) * 128],
                            rt[:, c * WC + mm * 512 : c * WC + (mm + 1) * 512],
                            start=True, stop=True)
                        if mm == 1:
                            inst.then_inc(peb1)
                        elif mm == 2:
                            inst.then_inc(pe_sem)

        @block.scalar
        def _(s):
            for j in range(L):
                for m in meta:
                    if m["path"] == "D":
                        continue
                    k, par, i = m["k"], m["par"], m["iz"]
                    if i >= NEB or j > 0:
                        # ebuf slot free once the tree NEB back read it
                        base = NZ * j + (-NZ if i < NEB else 0)
                        s.wait_ge(a2rd, base + (i - NEB) + 1)
                    dst = ebuf[i % NEB]
                    if ONEPIECE:
                        s.wait_ge(pe_sem, NK * j + k + 1)
                        s.copy(dst, pts[par]).then_inc(actB[par])
                    else:
                        # chase PE: banks 0-1 after its m1, then bank 2
                        if CHASE:
                            s.wait_ge(peb1, NK * j + k + 1)
                        else:
                            s.wait_ge(pe_sem, NK * j + k + 1)
                        s.copy(dst[:, 0:1024], pts[par][:, 0:1024]).then_inc(actA[par])
                        s.wait_ge(pe_sem, NK * j + k + 1)
                        s.copy(dst[:, 1024:WC], pts[par][:, 1024:WC]).then_inc(actB[par])

        @block.vector
        def _(v):
            for j in range(L):
                queue = []

                def emit_D(m, j=j):
                    k, par = m["k"], m["par"]

                    def go():
                        v.wait_ge(pe_sem, NK * j + k + 1)
                        v.tensor_reduce(
                            s1buf[:, k * G : (k + 1) * G], _grp(pts[par], 96),
                            axis=AXX, op=MIN).then_inc(dveF[par])
                    return go

                def emit_Ztree(ms, j=j):
                    # fused tree over 1 or 2 Z tiles; for 2, their ebuf
                    # slots are consecutive (iz even,odd with same %NEB
                    # pairing guaranteed by NEB even)
                    n = len(ms)
                    last = ms[-1]
                    i0 = ms[0]["iz"] % NEB
                    src = ebuf_all[:, i0 * WC : (i0 + n) * WC]
                    b48 = e48[(ms[0]["iz"] // 2) % 2][:, 0 : n * G * 48]
                    b24 = e24[(ms[0]["iz"] // 2) % 2][:, 0 : n * G * 24]
                    b12 = e12[(ms[0]["iz"] // 2) % 2][:, 0 : n * G * 12]
                    # output columns: per-tile 16-col blocks (uniform
                    # stride only if adjacent; handle via separate reduces)

                    def go():
                        v.wait_ge(actB[last["par"]],
                                  NAPAR[last["par"]] * j + last["iap"] + 1)
                        sg = _grp(src, 96)
                        v.tensor_tensor(
                            _grp(b48, 48), sg[:, :, 0:48], sg[:, :, 48:96],
                            op=MIN).then_inc(a2rd)
                        t2 = v.tensor_tensor(
                            _grp(b24, 24), _grp(b48, 48)[:, :, 0:24],
                            _grp(b48, 48)[:, :, 24:48], op=MIN)
                        if n == 2:
                            # second ebuf slot is also done after tt48, but
                            # the free must ride on a compute inst so it
                            # fires after the engine drains
                            t2.then_inc(a2rd)
                        v.tensor_tensor(
                            _grp(b12, 12), _grp(b24, 24)[:, :, 0:12],
                            _grp(b24, 24)[:, :, 12:24], op=MIN)
                        for idx, m in enumerate(ms):
                            k = m["k"]
                            r = v.tensor_reduce(
                                s1buf[:, k * G : (k + 1) * G],
                                _grp(b12[:, idx * G * 12 : (idx + 1) * G * 12], 12),
                                axis=AXX, op=MIN)
                        if last["k"] in fin_marker:
                            r.then_inc(fin_sem)
                    return go

                for m in meta:
                    if m["path"] == "D":
                        queue.append((emit_pos(m) + (0,), emit_D(m)))
                for g in groups:
                    queue.append((emit_pos(g[0]) + (0,), emit_Ztree(g)))
                queue.sort(key=lambda q: q[0])
                if j > 0:
                    v.wait_ge(out_sem, 48 * j)
                for _, go in queue:
                    go()

    _cache[("nc", L)] = nc
    return nc


def _prep_inputs(v1s, v2s, rid_to_vid):
    """Build per-core fused lhsT|rhs feature matrices."""
    g1 = v1s[:, rid_to_vid, :]  # [B, R, VR, 3]
    g2 = v2s[:, rid_to_vid, :]
    g1_64 = g1.astype(np.float64)
    g2_64 = g2.astype(np.float64)
    sq1 = (g1_64 * g1_64).sum(-1)  # [B, R, VR]
    sq2 = (g2_64 * g2_64).sum(-1)

    in_maps = []
    for core in range(NCORES):
        b, h = divmod(core, 2)
        rs = slice(RH * h, RH * (h + 1))
        a = np.empty((K, V + W), np.float32)
        a[0:3, 0:V] = -2.0 * g1[b, rs].reshape(V, 3).T
        a[3, 0:V] = sq1[b, rs].reshape(V).astype(np.float32)
        a[4, 0:V] = 1.0
        a[0:3, V:] = g2[b].reshape(W, 3).T
        a[3, V:] = 1.0
        a[4, V:] = sq2[b].reshape(W).astype(np.float32)
        in_maps.append({"ab": a})
    return in_maps


def kernel(v1s, v2s, cmaps, rid_to_vid):
    v1s = np.asarray(v1s)
    v2s = np.asarray(v2s)
    cmaps = np.asarray(cmaps)
    rid_to_vid = np.asarray(rid_to_vid)

    nc = _build()
    in_maps = _prep_inputs(v1s, v2s, rid_to_vid)
    res = run_bass_kernel_spmd(nc, in_maps, core_ids=list(range(NCORES)))

    # assemble [B, R, R] min squared distances (r = person1 region rows)
    md2 = np.empty((B, R, R), np.float32)
    for core in range(NCORES):
        b, h = divmod(core, 2)
        out = res.results[core]["s1out"]  # [128, NK*G], k = c*T + t
        # (p, c, t, g) -> v = t*128 + p, s = c*G + g
        per_v = out.reshape(128, NWC, T, G).transpose(2, 0, 1, 3).reshape(V, R)
        # segmented min over each region's 96 rows
        md2[b, RH * h : RH * (h + 1), :] = per_v.reshape(RH, VR, R).min(axis=1)

    md = np.sqrt(np.maximum(md2, 0.0))
    m = cmaps.astype(np.float32)
    return ((md * m).sum(axis=(1, 2)) / m.sum(axis=(1, 2))).astype(np.float32)


# revision 4
# speedup vs baseline: 1.0283x; 1.0132x over previous
"""Trainium2 Bass kernel for nn_ContactMapDistError — v3 (DVE+ACT reduce).

Computes, for each batch element b:
    mean over active contact pairs (r,s) of
      min_{v in region r, w in region s} || g1[b,r,v] - g2[b,s,w] ||

Strategy
--------
Host (cheap, O(B*R*VR)):
  - gather region vertex subsets g1, g2 via rid_to_vid
  - build K=5 feature matrices so one matmul yields pairwise squared
    distances: d2(v,w) = [-2x,-2y,-2z,sq1,1]_v . [x',y',z',1,sq2]_w
  - finish the v-axis min (segmented, tiny), sqrt, contact-mask mean

Device (8 cores SPMD; core i -> batch i//2, r-half i%2):
  - PE: fp32r matmuls fill [128, 1536] PSUM tiles (3 banks each, 2-slot
    parity ring), one 512-col matmul per bank.
  - The w-axis min (96 -> 1 per s-region) is the bottleneck: DVE
    tensor_reduce alone is ~93us busy vs PE ~35us. Hardware limits the
    options: GPSIMD has no PSUM port and no generic elementwise ops in
    this toolchain; tensor_tensor cannot read two PSUM operands; ScalarE
    has no min. So tiles are split between two paths:
      D: DVE grouped tensor_reduce straight from PSUM      (DVE 1725ns)
      Z: ACT copies the tile PSUM->SBUF bf16 in 2 bank-aligned pieces
         (chasing PE, early per-bank PSUM frees), then DVE runs a bf16
         2x_1p tensor_tensor min-tree + final reduce; trees of adjacent
         Z-tile pairs are fused to amortize per-inst overheads
                                     (ACT ~1566ns, DVE ~1020-1140ns)
    The 18/36 mix balances DVE (~66us busy) and ACT (~60us busy).
  - every instruction carries at most ONE semaphore update (ISA limit)
  - SP: split input DMA + 3 overlapped output drains (HWDGE)
"""

import sys

sys.path.insert(0, "/opt/trn_rl_repo")

import numpy as np

import concourse.bass as bass
import concourse.mybir as mybir
from concourse.bass_utils import run_bass_kernel_spmd

F32 = mybir.dt.float32
F32R = mybir.dt.float32r
BF16 = mybir.dt.bfloat16
MIN = mybir.AluOpType.min
AXX = mybir.AxisListType.X

B, N, R, VR = 4, 10475, 48, 96
NCORES = 8
RH = R // 2            # r-regions handled per core
V = RH * VR            # packed v columns per core = 2304
T = V // 128           # v-chunks of 128 partitions = 18
W = R * VR             # full w width = 4608
WC = 1536              # psum tile width (3 banks, 16 s-regions)
NWC = W // WC          # = 3
K = 5                  # contraction dim
NK = T * NWC           # total tile count = 54
G = WC // VR           # s-groups per tile = 16

# ---- tunables -------------------------------------------------------------
ND = 18                       # tiles on the D path (rest are Z)
DELTA_Z = 2                   # DVE-queue lag for Z-path trees
CHASE = True                  # ACT pieceA chases PE's m1 (else waits full)
ONEPIECE = False              # single full-tile ACT copy (less ACT busy,
                              # worse PSUM-ring overlap)
NEB = 6                       # ebuf ring (Z-path bf16 copies)
PAIR = True                   # fuse trees of adjacent Z-tile pairs
DRAIN_RANGES = [(0, 36), (36, 50), (50, 54)]   # tile ranges per output DMA

_cache = {}


def _mk_paths():
    """Spread D tiles evenly among the Z tiles (the 2-slot PSUM ring
    prefers alternating DVE- and ACT-consumed tiles). Tile 0 is forced
    to D so DVE's first reduce only waits for the first PSUM fill."""
    slots = ["D" if i * ND // NK > (i - 1) * ND // NK else "Z"
             for i in range(NK)]
    return slots


def _grp(ap, w):
    return ap.rearrange("p (g w) -> p g w", w=w)


def _build(L=1):
    if ("nc", L) in _cache:
        return _cache[("nc", L)]
    paths = _mk_paths()
    nc = bass.Bass()
    ab = nc.declare_dram_parameter("ab", [K, V + W], F32R, isOutput=False)
    s1out = nc.declare_dram_parameter("s1out", [128, NK * G], F32, isOutput=True)

    abt = nc.alloc_sbuf_tensor("abt", [K, V + W], F32R).ap()
    s1buf = nc.alloc_sbuf_tensor("s1buf", [128, NK * G], F32).ap()
    pts = [nc.alloc_psum_tensor(f"pt{i}", [128, WC], F32).ap() for i in range(2)]

    # Z-path scratch (bf16). ebuf slots are allocated as one tensor so a
    # fused pair-tree can address two consecutive slots with one AP.
    ebuf_all = nc.alloc_sbuf_tensor("eb", [128, NEB * WC], BF16).ap()
    ebuf = [ebuf_all[:, i * WC : (i + 1) * WC] for i in range(NEB)]
    e48 = [nc.alloc_sbuf_tensor(f"e48_{i}", [128, 2 * G * 48], BF16).ap() for i in range(2)]
    e24 = [nc.alloc_sbuf_tensor(f"e24_{i}", [128, 2 * G * 24], BF16).ap() for i in range(2)]
    e12 = [nc.alloc_sbuf_tensor(f"e12_{i}", [128, 2 * G * 12], BF16).ap() for i in range(2)]

    lt = abt[:, 0:V]
    rt = abt[:, V : V + W]

    # ---- static schedule bookkeeping ----
    # tile order is c-outer (k = c*T + t) so the first input DMA pieces
    # (lhsT + first rhs chunk) unblock the whole first third.
    meta = []
    iz = 0
    actpar = [0, 0]
    dpar = [0, 0]
    for k in range(NK):
        p = paths[k]
        c, t = divmod(k, T)
        par = k % 2
        m = {"path": p, "k": k, "c": c, "t": t, "par": par}
        if p == "D":
            m["idp"] = dpar[par]
            dpar[par] += 1
        else:
            m["iap"] = actpar[par]
            actpar[par] += 1
            m["iz"] = iz
            iz += 1
        meta.append(m)
    NZ = iz
    NDPAR = list(dpar)
    NAPAR = list(actpar)
    zs = [m for m in meta if m["path"] == "Z"]

    # pair up Z tiles whose ebuf slots are adjacent (even iz with its
    # successor) for fused trees; leftover tiles get a solo tree.
    groups = []           # list of [m] or [m1, m2]
    if PAIR:
        i = 0
        while i < NZ:
            if i + 1 < NZ:
                groups.append([zs[i], zs[i + 1]])
                i += 2
            else:
                groups.append([zs[i]])
                i += 1
    else:
        groups = [[m] for m in zs]
    for gi, grp_ms in enumerate(groups):
        for m in grp_ms:
            m["grp"] = gi

    # DVE queue emit position of each tile's final s1buf write
    def emit_pos(m):
        if m["path"] == "Z":
            g = groups[m["grp"]]
            return (g[-1]["k"] + DELTA_Z, 1)
        return (m["k"], 0)

    fin_marker = {}
    for third, (k0, k1) in enumerate(DRAIN_RANGES):
        last = max((m for m in meta if k0 <= m["k"] < k1), key=emit_pos)
        assert last["path"] == "Z", (
            "drain-range boundary must land on a Z tile so the fin inc can "
            "ride its final reduce")
        # the inc rides the group's final reduce; key by the group's last k
        fin_marker[groups[last["grp"]][-1]["k"]] = third

    with (
        nc.Block() as block,
        nc.semaphore("dma_in") as dma_in,
        nc.semaphore("dma_in2") as dma_in2,
        nc.semaphore("dma_in3") as dma_in3,
        nc.semaphore("pe_sem") as pe_sem,
        nc.semaphore("peb1") as peb1,
        nc.semaphore("dveF0") as dveF0,
        nc.semaphore("dveF1") as dveF1,
        nc.semaphore("actA0") as actA0,
        nc.semaphore("actA1") as actA1,
        nc.semaphore("actB0") as actB0,
        nc.semaphore("actB1") as actB1,
        nc.semaphore("a2rd") as a2rd,
        nc.semaphore("fin_sem") as fin_sem,
        nc.semaphore("out_sem") as out_sem,
    ):
        dveF = [dveF0, dveF1]
        actA = [actA0, actA1]
        actB = [actB0, actB1]

        @block.sync
        def _(sp):
            for j in range(L):
                if j > 0:
                    sp.wait_ge(pe_sem, NK * j)
                # piece 1: all of lhsT + first bank of rhs chunk 0 (m0 of
                # tile 0 can start); piece 2: rest of rhs chunk 0; piece 3:
                # rest. Separate semaphores: DMA completions are unordered.
                sp.dma_start(abt[:, 0 : V + 512],
                             ab[:, 0 : V + 512]).then_inc(dma_in, 16)
                sp.dma_start(abt[:, V + 512 : V + WC],
                             ab[:, V + 512 : V + WC]).then_inc(dma_in3, 16)
                sp.dma_start(abt[:, V + WC :], ab[:, V + WC :]).then_inc(dma_in2, 16)
                for third, (k0, k1) in enumerate(DRAIN_RANGES):
                    cols = slice(k0 * G, k1 * G)
                    sp.wait_ge(fin_sem, 3 * j + third + 1)
                    sp.dma_start(s1out[:, cols], s1buf[:, cols]).then_inc(out_sem, 16)
            sp.wait_ge(out_sem, 48 * L)

        @block.tensor
        def _(pe):
            for j in range(L):
                for m in meta:
                    k, par, c, t = m["k"], m["par"], m["c"], m["t"]
                    if k == 0:
                        pe.wait_ge(dma_in, 16 * (j + 1))
                    elif k == T:
                        pe.wait_ge(dma_in2, 16 * (j + 1))
                    pt = pts[par]
                    # refill gating: wait on the consumer of tile k-2
                    # (same parity slot); at iteration start, on the last
                    # same-parity tile of the previous iteration
                    if k >= 2:
                        pv, joff = meta[k - 2], 0
                    elif j > 0:
                        pv, joff = meta[NK - 2 + k], -1
                    else:
                        pv = None
                    for mm in range(3):
                        if k == 0 and mm == 1:
                            pe.wait_ge(dma_in3, 16 * (j + 1))
                        if pv is not None:
                            if pv["path"] == "D":
                                if mm == 0:
                                    pe.wait_ge(
                                        dveF[par],
                                        NDPAR[par] * (j + joff) + pv["idp"] + 1)
                            else:
                                if mm == 0:
                                    pe.wait_ge(
                                        actB[par] if ONEPIECE else actA[par],
                                        NAPAR[par] * (j + joff) + pv["iap"] + 1)
                                elif mm == 2 and not ONEPIECE:
                                    pe.wait_ge(
                                        actB[par],
                                        NAPAR[par] * (j + joff) + pv["iap"] + 1)
                        inst = pe.matmul(
                            pt[:, mm * 512 : (mm + 1) * 512],
                            lt[:, t * 128 : (t + 1) * 128],
                            rt[:, c * WC + mm * 512 : c * WC + (mm + 1) * 512],
                            start=True, stop=True)
                        if mm == 1:
                            inst.then_inc(peb1)
                        elif mm == 2:
                            inst.then_inc(pe_sem)

        @block.scalar
        def _(s):
            for j in range(L):
                for m in meta:
                    if m["path"] == "D":
                        continue
                    k, par, i = m["k"], m["par"], m["iz"]
                    if i >= NEB or j > 0:
                        # ebuf slot free once the tree NEB back read it
                        base = NZ * j + (-NZ if i < NEB else 0)
                        s.wait_ge(a2rd, base + (i - NEB) + 1)
                    dst = ebuf[i % NEB]
                    if ONEPIECE:
                        s.wait_ge(pe_sem, NK * j + k + 1)
                        s.copy(dst, pts[par]).then_inc(actB[par])
                    else:
                        # chase PE: banks 0-1 after its m1, then bank 2
                        if CHASE:
                            s.wait_ge(peb1, NK * j + k + 1)
                        else:
                            s.wait_ge(pe_sem, NK * j + k + 1)
                        s.copy(dst[:, 0:1024], pts[par][:, 0:1024]).then_inc(actA[par])
                        s.wait_ge(pe_sem, NK * j + k + 1)
                        s.copy(dst[:, 1024:WC], pts[par][:, 1024:WC]).then_inc(actB[par])

        @block.vector
        def _(v):
            for j in range(L):
                queue = []

                def emit_D(m, j=j):
                    k, par = m["k"], m["par"]

                    def go():
                        v.wait_ge(pe_sem, NK * j + k + 1)
                        v.tensor_reduce(
                            s1buf[:, k * G : (k + 1) * G], _grp(pts[par], 96),
                            axis=AXX, op=MIN).then_inc(dveF[par])
                    return go

                def emit_Ztree(ms, j=j):
                    # fused tree over 1 or 2 Z tiles; for 2, their ebuf
                    # slots are consecutive (iz even,odd with same %NEB
                    # pairing guaranteed by NEB even)
                    n = len(ms)
                    last = ms[-1]
                    i0 = ms[0]["iz"] % NEB
                    src = ebuf_all[:, i0 * WC : (i0 + n) * WC]
                    b48 = e48[(ms[0]["iz"] // 2) % 2][:, 0 : n * G * 48]
                    b24 = e24[(ms[0]["iz"] // 2) % 2][:, 0 : n * G * 24]
                    b12 = e12[(ms[0]["iz"] // 2) % 2][:, 0 : n * G * 12]
                    # output columns: per-tile 16-col blocks (uniform
                    # stride only if adjacent; handle via separate reduces)

                    def go():
                        v.wait_ge(actB[last["par"]],
                                  NAPAR[last["par"]] * j + last["iap"] + 1)
                        sg = _grp(src, 96)
                        v.tensor_tensor(
                            _grp(b48, 48), sg[:, :, 0:48], sg[:, :, 48:96],
                            op=MIN).then_inc(a2rd)
                        t2 = v.tensor_tensor(
                            _grp(b24, 24), _grp(b48, 48)[:, :, 0:24],
                            _grp(b48, 48)[:, :, 24:48], op=MIN)
                        if n == 2:
                            # second ebuf slot is also done after tt48, but
                            # the free must ride on a compute inst so it
                            # fires after the engine drains
                            t2.then_inc(a2rd)
                        v.tensor_tensor(
                            _grp(b12, 12), _grp(b24, 24)[:, :, 0:12],
                            _grp(b24, 24)[:, :, 12:24], op=MIN)
                        if n == 2 and ms[1]["k"] == ms[0]["k"] + 1:
                            # adjacent tiles: one fused reduce into the
                            # contiguous 32-col s1buf block
                            k = ms[0]["k"]
                            r = v.tensor_reduce(
                                s1buf[:, k * G : (k + 2) * G], _grp(b12, 12),
                                axis=AXX, op=MIN)
                        else:
                            for idx, m in enumerate(ms):
                                k = m["k"]
                                r = v.tensor_reduce(
                                    s1buf[:, k * G : (k + 1) * G],
                                    _grp(b12[:, idx * G * 12 : (idx + 1) * G * 12], 12),
                                    axis=AXX, op=MIN)
                        if last["k"] in fin_marker:
                            r.then_inc(fin_sem)
                    return go

                for m in meta:
                    if m["path"] == "D":
                        queue.append((emit_pos(m) + (0,), emit_D(m)))
                for g in groups:
                    queue.append((emit_pos(g[0]) + (0,), emit_Ztree(g)))
                queue.sort(key=lambda q: q[0])
                if j > 0:
                    v.wait_ge(out_sem, 48 * j)
                for _, go in queue:
                    go()

    _cache[("nc", L)] = nc
    return nc


def _prep_inputs(v1s, v2s, rid_to_vid):
    """Build per-core fused lhsT|rhs feature matrices."""
    g1 = v1s[:, rid_to_vid, :]  # [B, R, VR, 3]
    g2 = v2s[:, rid_to_vid, :]
    g1_64 = g1.astype(np.float64)
    g2_64 = g2.astype(np.float64)
    sq1 = (g1_64 * g1_64).sum(-1)  # [B, R, VR]
    sq2 = (g2_64 * g2_64).sum(-1)

    in_maps = []
    for core in range(NCORES):
        b, h = divmod(core, 2)
        rs = slice(RH * h, RH * (h + 1))
        a = np.empty((K, V + W), np.float32)
        a[0:3, 0:V] = -2.0 * g1[b, rs].reshape(V, 3).T
        a[3, 0:V] = sq1[b, rs].reshape(V).astype(np.float32)
        a[4, 0:V] = 1.0
        a[0:3, V:] = g2[b].reshape(W, 3).T
        a[3, V:] = 1.0
        a[4, V:] = sq2[b].reshape(W).astype(np.float32)
        in_maps.append({"ab": a})
    return in_maps


def kernel(v1s, v2s, cmaps, rid_to_vid):
    v1s = np.asarray(v1s)
    v2s = np.asarray(v2s)
    cmaps = np.asarray(cmaps)
    rid_to_vid = np.asarray(rid_to_vid)

    nc = _build()
    in_maps = _prep_inputs(v1s, v2s, rid_to_vid)
    res = run_bass_kernel_spmd(nc, in_maps, core_ids=list(range(NCORES)))

    # assemble [B, R, R] min squared distances (r = person1 region rows)
    md2 = np.empty((B, R, R), np.float32)
    for core in range(NCORES):
        b, h = divmod(core, 2)
        out = res.results[core]["s1out"]  # [128, NK*G], k = c*T + t
        # (p, c, t, g) -> v = t*128 + p, s = c*G + g
        per_v = out.reshape(128, NWC, T, G).transpose(2, 0, 1, 3).reshape(V, R)
        # segmented min over each region's 96 rows
        md2[b, RH * h : RH * (h + 1), :] = per_v.reshape(RH, VR, R).min(axis=1)

    md = np.sqrt(np.maximum(md2, 0.0))
    m = cmaps.astype(np.float32)
    return ((md * m).sum(axis=(1, 2)) / m.sum(axis=(1, 2))).astype(np.float32)


# revision 5
# speedup vs baseline: 1.0374x; 1.0089x over previous
"""Trainium2 Bass kernel for nn_ContactMapDistError — v3 (DVE+ACT reduce).

Computes, for each batch element b:
    mean over active contact pairs (r,s) of
      min_{v in region r, w in region s} || g1[b,r,v] - g2[b,s,w] ||

Strategy
--------
Host (cheap, O(B*R*VR)):
  - gather region vertex subsets g1, g2 via rid_to_vid
  - build K=5 feature matrices so one matmul yields pairwise squared
    distances: d2(v,w) = [-2x,-2y,-2z,sq1,1]_v . [x',y',z',1,sq2]_w
  - finish the v-axis min (segmented, tiny), sqrt, contact-mask mean

Device (8 cores SPMD; core i -> batch i//2, r-half i%2):
  - PE: fp32r matmuls fill [128, 1536] PSUM tiles (3 banks each, 2-slot
    parity ring), one 512-col matmul per bank.
  - The w-axis min (96 -> 1 per s-region) is the bottleneck: DVE
    tensor_reduce alone is ~93us busy vs PE ~35us. Hardware limits the
    options: GPSIMD has no PSUM port and no generic elementwise ops in
    this toolchain; tensor_tensor cannot read two PSUM operands; ScalarE
    has no min. So tiles are split between two paths:
      D: DVE grouped tensor_reduce straight from PSUM      (DVE 1725ns)
      Z: ACT copies the tile PSUM->SBUF bf16 in 2 bank-aligned pieces
         (chasing PE, early per-bank PSUM frees), then DVE runs a bf16
         2x_1p tensor_tensor min-tree + final reduce; trees of adjacent
         Z-tile pairs are fused to amortize per-inst overheads
                                     (ACT ~1566ns, DVE ~1020-1140ns)
    The 18/36 mix balances DVE (~66us busy) and ACT (~60us busy).
  - every instruction carries at most ONE semaphore update (ISA limit)
  - SP: split input DMA + 3 overlapped output drains (HWDGE)
"""

import sys

sys.path.insert(0, "/opt/trn_rl_repo")

import numpy as np

import concourse.bass as bass
import concourse.mybir as mybir
from concourse.bass_utils import run_bass_kernel_spmd

F32 = mybir.dt.float32
F32R = mybir.dt.float32r
BF16 = mybir.dt.bfloat16
MIN = mybir.AluOpType.min
AXX = mybir.AxisListType.X

B, N, R, VR = 4, 10475, 48, 96
NCORES = 8
RH = R // 2            # r-regions handled per core
V = RH * VR            # packed v columns per core = 2304
T = V // 128           # v-chunks of 128 partitions = 18
W = R * VR             # full w width = 4608
WC = 1536              # psum tile width (3 banks, 16 s-regions)
NWC = W // WC          # = 3
K = 5                  # contraction dim
NK = T * NWC           # total tile count = 54
G = WC // VR           # s-groups per tile = 16

# ---- tunables -------------------------------------------------------------
ND = 18                       # tiles on the D path (rest are Z)
HEAD_DD = 2                   # leading back-to-back D tiles (DVE warmup)
DELTA_Z = 2                   # DVE-queue lag for Z-path trees
CHASE = True                  # ACT pieceA chases PE's m1 (else waits full)
ONEPIECE = False              # single full-tile ACT copy (less ACT busy,
                              # worse PSUM-ring overlap)
NEB = 6                       # ebuf ring (Z-path bf16 copies)
PAIR = True                   # fuse trees of adjacent Z-tile pairs
DRAIN_RANGES = [(0, 36), (36, 50), (50, 54)]   # tile ranges per output DMA

_cache = {}


def _mk_paths():
    """Spread D tiles evenly among the Z tiles (the 2-slot PSUM ring
    prefers alternating DVE- and ACT-consumed tiles). The first HEAD_DD
    tiles are forced to D so DVE starts as soon as the first PSUM fills
    land, covering the copy-pipeline warmup."""
    slots = ["D" if i * ND // NK > (i - 1) * ND // NK else "Z"
             for i in range(NK)]
    return slots


def _grp(ap, w):
    return ap.rearrange("p (g w) -> p g w", w=w)


def _build(L=1):
    if ("nc", L) in _cache:
        return _cache[("nc", L)]
    paths = _mk_paths()
    nc = bass.Bass()
    ab = nc.declare_dram_parameter("ab", [K, V + W], F32R, isOutput=False)
    s1out = nc.declare_dram_parameter("s1out", [128, NK * G], F32, isOutput=True)

    abt = nc.alloc_sbuf_tensor("abt", [K, V + W], F32R).ap()
    s1buf = nc.alloc_sbuf_tensor("s1buf", [128, NK * G], F32).ap()
    pts = [nc.alloc_psum_tensor(f"pt{i}", [128, WC], F32).ap() for i in range(2)]

    # Z-path scratch (bf16). ebuf slots are allocated as one tensor so a
    # fused pair-tree can address two consecutive slots with one AP.
    ebuf_all = nc.alloc_sbuf_tensor("eb", [128, NEB * WC], BF16).ap()
    ebuf = [ebuf_all[:, i * WC : (i + 1) * WC] for i in range(NEB)]
    e48 = [nc.alloc_sbuf_tensor(f"e48_{i}", [128, 2 * G * 48], BF16).ap() for i in range(2)]
    e24 = [nc.alloc_sbuf_tensor(f"e24_{i}", [128, 2 * G * 24], BF16).ap() for i in range(2)]
    e12 = [nc.alloc_sbuf_tensor(f"e12_{i}", [128, 2 * G * 12], BF16).ap() for i in range(2)]
    e6 = [nc.alloc_sbuf_tensor(f"e6_{i}", [128, 2 * G * 6], BF16).ap() for i in range(2)]

    lt = abt[:, 0:V]
    rt = abt[:, V : V + W]

    # ---- static schedule bookkeeping ----
    # tile order is c-outer (k = c*T + t) so the first input DMA pieces
    # (lhsT + first rhs chunk) unblock the whole first third.
    meta = []
    iz = 0
    actpar = [0, 0]
    dpar = [0, 0]
    for k in range(NK):
        p = paths[k]
        c, t = divmod(k, T)
        par = k % 2
        m = {"path": p, "k": k, "c": c, "t": t, "par": par}
        if p == "D":
            m["idp"] = dpar[par]
            dpar[par] += 1
        else:
            m["iap"] = actpar[par]
            actpar[par] += 1
            m["iz"] = iz
            iz += 1
        meta.append(m)
    NZ = iz
    NDPAR = list(dpar)
    NAPAR = list(actpar)
    zs = [m for m in meta if m["path"] == "Z"]

    # pair up Z tiles whose ebuf slots are adjacent (even iz with its
    # successor) for fused trees; leftover tiles get a solo tree.
    groups = []           # list of [m] or [m1, m2]
    if PAIR:
        i = 0
        while i < NZ:
            if i + 1 < NZ:
                groups.append([zs[i], zs[i + 1]])
                i += 2
            else:
                groups.append([zs[i]])
                i += 1
    else:
        groups = [[m] for m in zs]
    for gi, grp_ms in enumerate(groups):
        for m in grp_ms:
            m["grp"] = gi

    # DVE queue emit position of each tile's final s1buf write
    def emit_pos(m):
        if m["path"] == "Z":
            g = groups[m["grp"]]
            return (g[-1]["k"] + DELTA_Z, 1)
        return (m["k"], 0)

    fin_marker = {}
    for third, (k0, k1) in enumerate(DRAIN_RANGES):
        last = max((m for m in meta if k0 <= m["k"] < k1), key=emit_pos)
        assert last["path"] == "Z", (
            "drain-range boundary must land on a Z tile so the fin inc can "
            "ride its final reduce")
        # the inc rides the group's final reduce; key by the group's last k
        fin_marker[groups[last["grp"]][-1]["k"]] = third

    with (
        nc.Block() as block,
        nc.semaphore("dma_in") as dma_in,
        nc.semaphore("dma_in2") as dma_in2,
        nc.semaphore("dma_in3") as dma_in3,
        nc.semaphore("pe_sem") as pe_sem,
        nc.semaphore("peb1") as peb1,
        nc.semaphore("dveF0") as dveF0,
        nc.semaphore("dveF1") as dveF1,
        nc.semaphore("actA0") as actA0,
        nc.semaphore("actA1") as actA1,
        nc.semaphore("actB0") as actB0,
        nc.semaphore("actB1") as actB1,
        nc.semaphore("a2rd") as a2rd,
        nc.semaphore("fin_sem") as fin_sem,
        nc.semaphore("out_sem") as out_sem,
    ):
        dveF = [dveF0, dveF1]
        actA = [actA0, actA1]
        actB = [actB0, actB1]

        @block.sync
        def _(sp):
            for j in range(L):
                if j > 0:
                    sp.wait_ge(pe_sem, NK * j)
                # piece 1: all of lhsT + first bank of rhs chunk 0 (m0 of
                # tile 0 can start); piece 2: rest of rhs chunk 0; piece 3:
                # rest. Separate semaphores: DMA completions are unordered.
                sp.dma_start(abt[:, 0 : V + 512],
                             ab[:, 0 : V + 512]).then_inc(dma_in, 16)
                sp.dma_start(abt[:, V + 512 : V + WC],
                             ab[:, V + 512 : V + WC]).then_inc(dma_in3, 16)
                sp.dma_start(abt[:, V + WC :], ab[:, V + WC :]).then_inc(dma_in2, 16)
                for third, (k0, k1) in enumerate(DRAIN_RANGES):
                    cols = slice(k0 * G, k1 * G)
                    sp.wait_ge(fin_sem, 3 * j + third + 1)
                    sp.dma_start(s1out[:, cols], s1buf[:, cols]).then_inc(out_sem, 16)
            sp.wait_ge(out_sem, 48 * L)

        @block.tensor
        def _(pe):
            for j in range(L):
                for m in meta:
                    k, par, c, t = m["k"], m["par"], m["c"], m["t"]
                    if k == 0:
                        pe.wait_ge(dma_in, 16 * (j + 1))
                    elif k == T:
                        pe.wait_ge(dma_in2, 16 * (j + 1))
                    pt = pts[par]
                    # refill gating: wait on the consumer of tile k-2
                    # (same parity slot); at iteration start, on the last
                    # same-parity tile of the previous iteration
                    if k >= 2:
                        pv, joff = meta[k - 2], 0
                    elif j > 0:
                        pv, joff = meta[NK - 2 + k], -1
                    else:
                        pv = None
                    for mm in range(3):
                        if k == 0 and mm == 1:
                            pe.wait_ge(dma_in3, 16 * (j + 1))
                        if pv is not None:
                            if pv["path"] == "D":
                                if mm == 0:
                                    pe.wait_ge(
                                        dveF[par],
                                        NDPAR[par] * (j + joff) + pv["idp"] + 1)
                            else:
                                if mm == 0:
                                    pe.wait_ge(
                                        actB[par] if ONEPIECE else actA[par],
                                        NAPAR[par] * (j + joff) + pv["iap"] + 1)
                                elif mm == 2 and not ONEPIECE:
                                    pe.wait_ge(
                                        actB[par],
                                        NAPAR[par] * (j + joff) + pv["iap"] + 1)
                        inst = pe.matmul(
                            pt[:, mm * 512 : (mm + 1) * 512],
                            lt[:, t * 128 : (t + 1) * 128],
                            rt[:, c * WC + mm * 512 : c * WC + (mm + 1) * 512],
                            start=True, stop=True)
                        if mm == 1:
                            inst.then_inc(peb1)
                        elif mm == 2:
                            inst.then_inc(pe_sem)

        @block.scalar
        def _(s):
            for j in range(L):
                for m in meta:
                    if m["path"] == "D":
                        continue
                    k, par, i = m["k"], m["par"], m["iz"]
                    if i >= NEB or j > 0:
                        # ebuf slot free once the tree NEB back read it
                        base = NZ * j + (-NZ if i < NEB else 0)
                        s.wait_ge(a2rd, base + (i - NEB) + 1)
                    dst = ebuf[i % NEB]
                    if ONEPIECE:
                        s.wait_ge(pe_sem, NK * j + k + 1)
                        s.copy(dst, pts[par]).then_inc(actB[par])
                    else:
                        # chase PE: banks 0-1 after its m1, then bank 2
                        if CHASE:
                            s.wait_ge(peb1, NK * j + k + 1)
                        else:
                            s.wait_ge(pe_sem, NK * j + k + 1)
                        s.copy(dst[:, 0:1024], pts[par][:, 0:1024]).then_inc(actA[par])
                        s.wait_ge(pe_sem, NK * j + k + 1)
                        s.copy(dst[:, 1024:WC], pts[par][:, 1024:WC]).then_inc(actB[par])

        @block.vector
        def _(v):
            for j in range(L):
                queue = []

                def emit_D(m, j=j):
                    k, par = m["k"], m["par"]

                    def go():
                        v.wait_ge(pe_sem, NK * j + k + 1)
                        v.tensor_reduce(
                            s1buf[:, k * G : (k + 1) * G], _grp(pts[par], 96),
                            axis=AXX, op=MIN).then_inc(dveF[par])
                    return go

                def emit_Ztree(ms, j=j):
                    # fused tree over 1 or 2 Z tiles; for 2, their ebuf
                    # slots are consecutive (iz even,odd with same %NEB
                    # pairing guaranteed by NEB even)
                    n = len(ms)
                    last = ms[-1]
                    i0 = ms[0]["iz"] % NEB
                    src = ebuf_all[:, i0 * WC : (i0 + n) * WC]
                    b48 = e48[(ms[0]["iz"] // 2) % 2][:, 0 : n * G * 48]
                    b24 = e24[(ms[0]["iz"] // 2) % 2][:, 0 : n * G * 24]
                    b12 = e12[(ms[0]["iz"] // 2) % 2][:, 0 : n * G * 12]
                    b6 = e6[(ms[0]["iz"] // 2) % 2][:, 0 : n * G * 6]
                    # output columns: per-tile 16-col blocks (uniform
                    # stride only if adjacent; handle via separate reduces)

                    def go():
                        v.wait_ge(actB[last["par"]],
                                  NAPAR[last["par"]] * j + last["iap"] + 1)
                        sg = _grp(src, 96)
                        v.tensor_tensor(
                            _grp(b48, 48), sg[:, :, 0:48], sg[:, :, 48:96],
                            op=MIN).then_inc(a2rd)
                        t2 = v.tensor_tensor(
                            _grp(b24, 24), _grp(b48, 48)[:, :, 0:24],
                            _grp(b48, 48)[:, :, 24:48], op=MIN)
                        if n == 2:
                            # second ebuf slot is also done after tt48, but
                            # the free must ride on a compute inst so it
                            # fires after the engine drains
                            t2.then_inc(a2rd)
                        v.tensor_tensor(
                            _grp(b12, 12), _grp(b24, 24)[:, :, 0:12],
                            _grp(b24, 24)[:, :, 12:24], op=MIN)
                        v.tensor_tensor(
                            _grp(b6, 6), _grp(b12, 12)[:, :, 0:6],
                            _grp(b12, 12)[:, :, 6:12], op=MIN)
                        if n == 2 and ms[1]["k"] == ms[0]["k"] + 1:
                            # adjacent tiles: one fused reduce into the
                            # contiguous 32-col s1buf block
                            k = ms[0]["k"]
                            r = v.tensor_reduce(
                                s1buf[:, k * G : (k + 2) * G], _grp(b6, 6),
                                axis=AXX, op=MIN)
                        else:
                            for idx, m in enumerate(ms):
                                k = m["k"]
                                r = v.tensor_reduce(
                                    s1buf[:, k * G : (k + 1) * G],
                                    _grp(b6[:, idx * G * 6 : (idx + 1) * G * 6], 6),
                                    axis=AXX, op=MIN)
                        if last["k"] in fin_marker:
                            r.then_inc(fin_sem)
                    return go

                for m in meta:
                    if m["path"] == "D":
                        queue.append((emit_pos(m) + (0,), emit_D(m)))
                for g in groups:
                    queue.append((emit_pos(g[0]) + (0,), emit_Ztree(g)))
                queue.sort(key=lambda q: q[0])
                if j > 0:
                    v.wait_ge(out_sem, 48 * j)
                for _, go in queue:
                    go()

    _cache[("nc", L)] = nc
    return nc


def _prep_inputs(v1s, v2s, rid_to_vid):
    """Build per-core fused lhsT|rhs feature matrices."""
    g1 = v1s[:, rid_to_vid, :]  # [B, R, VR, 3]
    g2 = v2s[:, rid_to_vid, :]
    g1_64 = g1.astype(np.float64)
    g2_64 = g2.astype(np.float64)
    sq1 = (g1_64 * g1_64).sum(-1)  # [B, R, VR]
    sq2 = (g2_64 * g2_64).sum(-1)

    in_maps = []
    for core in range(NCORES):
        b, h = divmod(core, 2)
        rs = slice(RH * h, RH * (h + 1))
        a = np.empty((K, V + W), np.float32)
        a[0:3, 0:V] = -2.0 * g1[b, rs].reshape(V, 3).T
        a[3, 0:V] = sq1[b, rs].reshape(V).astype(np.float32)
        a[4, 0:V] = 1.0
        a[0:3, V:] = g2[b].reshape(W, 3).T
        a[3, V:] = 1.0
        a[4, V:] = sq2[b].reshape(W).astype(np.float32)
        in_maps.append({"ab": a})
    return in_maps


def kernel(v1s, v2s, cmaps, rid_to_vid):
    v1s = np.asarray(v1s)
    v2s = np.asarray(v2s)
    cmaps = np.asarray(cmaps)
    rid_to_vid = np.asarray(rid_to_vid)

    nc = _build()
    in_maps = _prep_inputs(v1s, v2s, rid_to_vid)
    res = run_bass_kernel_spmd(nc, in_maps, core_ids=list(range(NCORES)))

    # assemble [B, R, R] min squared distances (r = person1 region rows)
    md2 = np.empty((B, R, R), np.float32)
    for core in range(NCORES):
        b, h = divmod(core, 2)
        out = res.results[core]["s1out"]  # [128, NK*G], k = c*T + t
        # (p, c, t, g) -> v = t*128 + p, s = c*G + g
        per_v = out.reshape(128, NWC, T, G).transpose(2, 0, 1, 3).reshape(V, R)
        # segmented min over each region's 96 rows
        md2[b, RH * h : RH * (h + 1), :] = per_v.reshape(RH, VR, R).min(axis=1)

    md = np.sqrt(np.maximum(md2, 0.0))
    m = cmaps.astype(np.float32)
    return ((md * m).sum(axis=(1, 2)) / m.sum(axis=(1, 2))).astype(np.float32)


# revision 6
# speedup vs baseline: 1.0386x; 1.0012x over previous
"""Trainium2 Bass kernel for nn_ContactMapDistError — v3 (DVE+ACT reduce).

Computes, for each batch element b:
    mean over active contact pairs (r,s) of
      min_{v in region r, w in region s} || g1[b,r,v] - g2[b,s,w] ||

Strategy
--------
Host (cheap, O(B*R*VR)):
  - gather region vertex subsets g1, g2 via rid_to_vid
  - build K=5 feature matrices so one matmul yields pairwise squared
    distances: d2(v,w) = [-2x,-2y,-2z,sq1,1]_v . [x',y',z',1,sq2]_w
  - finish the v-axis min (segmented, tiny), sqrt, contact-mask mean

Device (8 cores SPMD; core i -> batch i//2, r-half i%2):
  - PE: fp32r matmuls fill [128, 1536] PSUM tiles (3 banks each, 2-slot
    parity ring), one 512-col matmul per bank.
  - The w-axis min (96 -> 1 per s-region) is the bottleneck: DVE
    tensor_reduce alone is ~93us busy vs PE ~35us. Hardware limits the
    options: GPSIMD has no PSUM port and no generic elementwise ops in
    this toolchain; tensor_tensor cannot read two PSUM operands; ScalarE
    has no min. So tiles are split between two paths:
      D: DVE grouped tensor_reduce straight from PSUM      (DVE 1725ns)
      Z: ACT copies the tile PSUM->SBUF bf16 in 2 bank-aligned pieces
         (chasing PE, early per-bank PSUM frees), then DVE runs a bf16
         2x_1p tensor_tensor min-tree + final reduce; trees of adjacent
         Z-tile pairs are fused to amortize per-inst overheads
                                     (ACT ~1566ns, DVE ~1020-1140ns)
    The 18/36 mix balances DVE (~66us busy) and ACT (~60us busy).
  - every instruction carries at most ONE semaphore update (ISA limit)
  - SP: split input DMA + 3 overlapped output drains (HWDGE)
"""

import sys

sys.path.insert(0, "/opt/trn_rl_repo")

import numpy as np

import concourse.bass as bass
import concourse.mybir as mybir
from concourse.bass_utils import run_bass_kernel_spmd

F32 = mybir.dt.float32
F32R = mybir.dt.float32r
BF16 = mybir.dt.bfloat16
MIN = mybir.AluOpType.min
AXX = mybir.AxisListType.X

B, N, R, VR = 4, 10475, 48, 96
NCORES = 8
RH = R // 2            # r-regions handled per core
V = RH * VR            # packed v columns per core = 2304
T = V // 128           # v-chunks of 128 partitions = 18
W = R * VR             # full w width = 4608
WC = 1536              # psum tile width (3 banks, 16 s-regions)
NWC = W // WC          # = 3
K = 5                  # contraction dim
NK = T * NWC           # total tile count = 54
G = WC // VR           # s-groups per tile = 16

# ---- tunables -------------------------------------------------------------
ND = 18                       # tiles on the D path (rest are Z)
HEAD_DD = 2                   # leading back-to-back D tiles (DVE warmup)
DELTA_Z = 2                   # DVE-queue lag for Z-path trees
CHASE = True                  # ACT pieceA chases PE's m1 (else waits full)
ONEPIECE = False              # single full-tile ACT copy (less ACT busy,
                              # worse PSUM-ring overlap)
NEB = 6                       # ebuf ring (Z-path bf16 copies)
PAIR = True                   # fuse trees of adjacent Z-tile pairs
DRAIN_RANGES = [(0, 40), (40, 52), (52, 54)]   # tile ranges per output DMA

_cache = {}


def _mk_paths():
    """Spread D tiles evenly among the Z tiles (the 2-slot PSUM ring
    prefers alternating DVE- and ACT-consumed tiles). The first HEAD_DD
    tiles are forced to D so DVE starts as soon as the first PSUM fills
    land, covering the copy-pipeline warmup."""
    slots = ["D" if i * ND // NK > (i - 1) * ND // NK else "Z"
             for i in range(NK)]
    return slots


def _grp(ap, w):
    return ap.rearrange("p (g w) -> p g w", w=w)


def _build(L=1):
    if ("nc", L) in _cache:
        return _cache[("nc", L)]
    paths = _mk_paths()
    nc = bass.Bass()
    ab = nc.declare_dram_parameter("ab", [K, V + W], F32R, isOutput=False)
    s1out = nc.declare_dram_parameter("s1out", [128, NK * G], F32, isOutput=True)

    abt = nc.alloc_sbuf_tensor("abt", [K, V + W], F32R).ap()
    s1buf = nc.alloc_sbuf_tensor("s1buf", [128, NK * G], F32).ap()
    pts = [nc.alloc_psum_tensor(f"pt{i}", [128, WC], F32).ap() for i in range(2)]

    # Z-path scratch (bf16). ebuf slots are allocated as one tensor so a
    # fused pair-tree can address two consecutive slots with one AP.
    ebuf_all = nc.alloc_sbuf_tensor("eb", [128, NEB * WC], BF16).ap()
    ebuf = [ebuf_all[:, i * WC : (i + 1) * WC] for i in range(NEB)]
    e48 = [nc.alloc_sbuf_tensor(f"e48_{i}", [128, 2 * G * 48], BF16).ap() for i in range(2)]
    e24 = [nc.alloc_sbuf_tensor(f"e24_{i}", [128, 2 * G * 24], BF16).ap() for i in range(2)]
    e12 = [nc.alloc_sbuf_tensor(f"e12_{i}", [128, 2 * G * 12], BF16).ap() for i in range(2)]
    e6 = [nc.alloc_sbuf_tensor(f"e6_{i}", [128, 2 * G * 6], BF16).ap() for i in range(2)]

    lt = abt[:, 0:V]
    rt = abt[:, V : V + W]

    # ---- static schedule bookkeeping ----
    # tile order is c-outer (k = c*T + t) so the first input DMA pieces
    # (lhsT + first rhs chunk) unblock the whole first third.
    meta = []
    iz = 0
    actpar = [0, 0]
    dpar = [0, 0]
    for k in range(NK):
        p = paths[k]
        c, t = divmod(k, T)
        par = k % 2
        m = {"path": p, "k": k, "c": c, "t": t, "par": par}
        if p == "D":
            m["idp"] = dpar[par]
            dpar[par] += 1
        else:
            m["iap"] = actpar[par]
            actpar[par] += 1
            m["iz"] = iz
            iz += 1
        meta.append(m)
    NZ = iz
    NDPAR = list(dpar)
    NAPAR = list(actpar)
    zs = [m for m in meta if m["path"] == "Z"]

    # pair up Z tiles whose ebuf slots are adjacent (even iz with its
    # successor) for fused trees; leftover tiles get a solo tree.
    groups = []           # list of [m] or [m1, m2]
    if PAIR:
        i = 0
        while i < NZ:
            if i + 1 < NZ:
                groups.append([zs[i], zs[i + 1]])
                i += 2
            else:
                groups.append([zs[i]])
                i += 1
    else:
        groups = [[m] for m in zs]
    for gi, grp_ms in enumerate(groups):
        for m in grp_ms:
            m["grp"] = gi

    # DVE queue emit position of each tile's final s1buf write
    def emit_pos(m):
        if m["path"] == "Z":
            g = groups[m["grp"]]
            return (g[-1]["k"] + DELTA_Z, 1)
        return (m["k"], 0)

    fin_marker = {}
    for third, (k0, k1) in enumerate(DRAIN_RANGES):
        last = max((m for m in meta if k0 <= m["k"] < k1), key=emit_pos)
        assert last["path"] == "Z", (
            "drain-range boundary must land on a Z tile so the fin inc can "
            "ride its final reduce")
        # the inc rides the group's final reduce; key by the group's last k
        fin_marker[groups[last["grp"]][-1]["k"]] = third

    with (
        nc.Block() as block,
        nc.semaphore("dma_in") as dma_in,
        nc.semaphore("dma_in2") as dma_in2,
        nc.semaphore("dma_in3") as dma_in3,
        nc.semaphore("pe_sem") as pe_sem,
        nc.semaphore("peb1") as peb1,
        nc.semaphore("dveF0") as dveF0,
        nc.semaphore("dveF1") as dveF1,
        nc.semaphore("actA0") as actA0,
        nc.semaphore("actA1") as actA1,
        nc.semaphore("actB0") as actB0,
        nc.semaphore("actB1") as actB1,
        nc.semaphore("a2rd") as a2rd,
        nc.semaphore("fin_sem") as fin_sem,
        nc.semaphore("out_sem") as out_sem,
    ):
        dveF = [dveF0, dveF1]
        actA = [actA0, actA1]
        actB = [actB0, actB1]

        @block.sync
        def _(sp):
            for j in range(L):
                if j > 0:
                    sp.wait_ge(pe_sem, NK * j)
                # piece 1: all of lhsT + first bank of rhs chunk 0 (m0 of
                # tile 0 can start); piece 2: rest of rhs chunk 0; piece 3:
                # rest. Separate semaphores: DMA completions are unordered.
                sp.dma_start(abt[:, 0 : V + 512],
                             ab[:, 0 : V + 512]).then_inc(dma_in, 16)
                sp.dma_start(abt[:, V + 512 : V + WC],
                             ab[:, V + 512 : V + WC]).then_inc(dma_in3, 16)
                sp.dma_start(abt[:, V + WC :], ab[:, V + WC :]).then_inc(dma_in2, 16)
                for third, (k0, k1) in enumerate(DRAIN_RANGES):
                    cols = slice(k0 * G, k1 * G)
                    sp.wait_ge(fin_sem, 3 * j + third + 1)
                    sp.dma_start(s1out[:, cols], s1buf[:, cols]).then_inc(out_sem, 16)
            sp.wait_ge(out_sem, 48 * L)

        @block.tensor
        def _(pe):
            for j in range(L):
                for m in meta:
                    k, par, c, t = m["k"], m["par"], m["c"], m["t"]
                    if k == 0:
                        pe.wait_ge(dma_in, 16 * (j + 1))
                    elif k == T:
                        pe.wait_ge(dma_in2, 16 * (j + 1))
                    pt = pts[par]
                    # refill gating: wait on the consumer of tile k-2
                    # (same parity slot); at iteration start, on the last
                    # same-parity tile of the previous iteration
                    if k >= 2:
                        pv, joff = meta[k - 2], 0
                    elif j > 0:
                        pv, joff = meta[NK - 2 + k], -1
                    else:
                        pv = None
                    for mm in range(3):
                        if k == 0 and mm == 1:
                            pe.wait_ge(dma_in3, 16 * (j + 1))
                        if pv is not None:
                            if pv["path"] == "D":
                                if mm == 0:
                                    pe.wait_ge(
                                        dveF[par],
                                        NDPAR[par] * (j + joff) + pv["idp"] + 1)
                            else:
                                if mm == 0:
                                    pe.wait_ge(
                                        actB[par] if ONEPIECE else actA[par],
                                        NAPAR[par] * (j + joff) + pv["iap"] + 1)
                                elif mm == 2 and not ONEPIECE:
                                    pe.wait_ge(
                                        actB[par],
                                        NAPAR[par] * (j + joff) + pv["iap"] + 1)
                        inst = pe.matmul(
                            pt[:, mm * 512 : (mm + 1) * 512],
                            lt[:, t * 128 : (t + 1) * 128],
                            rt[:, c * WC + mm * 512 : c * WC + (mm + 1) * 512],
                            start=True, stop=True)
                        if mm == 1:
                            inst.then_inc(peb1)
                        elif mm == 2:
                            inst.then_inc(pe_sem)

        @block.scalar
        def _(s):
            for j in range(L):
                for m in meta:
                    if m["path"] == "D":
                        continue
                    k, par, i = m["k"], m["par"], m["iz"]
                    if i >= NEB or j > 0:
                        # ebuf slot free once the tree NEB back read it
                        base = NZ * j + (-NZ if i < NEB else 0)
                        s.wait_ge(a2rd, base + (i - NEB) + 1)
                    dst = ebuf[i % NEB]
                    if ONEPIECE:
                        s.wait_ge(pe_sem, NK * j + k + 1)
                        s.copy(dst, pts[par]).then_inc(actB[par])
                    else:
                        # chase PE: banks 0-1 after its m1, then bank 2
                        if CHASE:
                            s.wait_ge(peb1, NK * j + k + 1)
                        else:
                            s.wait_ge(pe_sem, NK * j + k + 1)
                        s.copy(dst[:, 0:1024], pts[par][:, 0:1024]).then_inc(actA[par])
                        s.wait_ge(pe_sem, NK * j + k + 1)
                        s.copy(dst[:, 1024:WC], pts[par][:, 1024:WC]).then_inc(actB[par])

        @block.vector
        def _(v):
            for j in range(L):
                queue = []

                def emit_D(m, j=j):
                    k, par = m["k"], m["par"]

                    def go():
                        v.wait_ge(pe_sem, NK * j + k + 1)
                        v.tensor_reduce(
                            s1buf[:, k * G : (k + 1) * G], _grp(pts[par], 96),
                            axis=AXX, op=MIN).then_inc(dveF[par])
                    return go

                def emit_Ztree(ms, j=j):
                    # fused tree over 1 or 2 Z tiles; for 2, their ebuf
                    # slots are consecutive (iz even,odd with same %NEB
                    # pairing guaranteed by NEB even)
                    n = len(ms)
                    last = ms[-1]
                    i0 = ms[0]["iz"] % NEB
                    src = ebuf_all[:, i0 * WC : (i0 + n) * WC]
                    b48 = e48[(ms[0]["iz"] // 2) % 2][:, 0 : n * G * 48]
                    b24 = e24[(ms[0]["iz"] // 2) % 2][:, 0 : n * G * 24]
                    b12 = e12[(ms[0]["iz"] // 2) % 2][:, 0 : n * G * 12]
                    b6 = e6[(ms[0]["iz"] // 2) % 2][:, 0 : n * G * 6]
                    # output columns: per-tile 16-col blocks (uniform
                    # stride only if adjacent; handle via separate reduces)

                    def go():
                        v.wait_ge(actB[last["par"]],
                                  NAPAR[last["par"]] * j + last["iap"] + 1)
                        sg = _grp(src, 96)
                        v.tensor_tensor(
                            _grp(b48, 48), sg[:, :, 0:48], sg[:, :, 48:96],
                            op=MIN).then_inc(a2rd)
                        t2 = v.tensor_tensor(
                            _grp(b24, 24), _grp(b48, 48)[:, :, 0:24],
                            _grp(b48, 48)[:, :, 24:48], op=MIN)
                        if n == 2:
                            # second ebuf slot is also done after tt48, but
                            # the free must ride on a compute inst so it
                            # fires after the engine drains
                            t2.then_inc(a2rd)
                        v.tensor_tensor(
                            _grp(b12, 12), _grp(b24, 24)[:, :, 0:12],
                            _grp(b24, 24)[:, :, 12:24], op=MIN)
                        v.tensor_tensor(
                            _grp(b6, 6), _grp(b12, 12)[:, :, 0:6],
                            _grp(b12, 12)[:, :, 6:12], op=MIN)
                        if n == 2 and ms[1]["k"] == ms[0]["k"] + 1:
                            # adjacent tiles: one fused reduce into the
                            # contiguous 32-col s1buf block
                            k = ms[0]["k"]
                            r = v.tensor_reduce(
                                s1buf[:, k * G : (k + 2) * G], _grp(b6, 6),
                                axis=AXX, op=MIN)
                        else:
                            for idx, m in enumerate(ms):
                                k = m["k"]
                                r = v.tensor_reduce(
                                    s1buf[:, k * G : (k + 1) * G],
                                    _grp(b6[:, idx * G * 6 : (idx + 1) * G * 6], 6),
                                    axis=AXX, op=MIN)
                        if last["k"] in fin_marker:
                            r.then_inc(fin_sem)
                    return go

                for m in meta:
                    if m["path"] == "D":
                        queue.append((emit_pos(m) + (0,), emit_D(m)))
                for g in groups:
                    queue.append((emit_pos(g[0]) + (0,), emit_Ztree(g)))
                queue.sort(key=lambda q: q[0])
                if j > 0:
                    v.wait_ge(out_sem, 48 * j)
                for _, go in queue:
                    go()

    _cache[("nc", L)] = nc
    return nc


def _prep_inputs(v1s, v2s, rid_to_vid):
    """Build per-core fused lhsT|rhs feature matrices."""
    g1 = v1s[:, rid_to_vid, :]  # [B, R, VR, 3]
    g2 = v2s[:, rid_to_vid, :]
    g1_64 = g1.astype(np.float64)
    g2_64 = g2.astype(np.float64)
    sq1 = (g1_64 * g1_64).sum(-1)  # [B, R, VR]
    sq2 = (g2_64 * g2_64).sum(-1)

    in_maps = []
    for core in range(NCORES):
        b, h = divmod(core, 2)
        rs = slice(RH * h, RH * (h + 1))
        a = np.empty((K, V + W), np.float32)
        a[0:3, 0:V] = -2.0 * g1[b, rs].reshape(V, 3).T
        a[3, 0:V] = sq1[b, rs].reshape(V).astype(np.float32)
        a[4, 0:V] = 1.0
        a[0:3, V:] = g2[b].reshape(W, 3).T
        a[3, V:] = 1.0
        a[4, V:] = sq2[b].reshape(W).astype(np.float32)
        in_maps.append({"ab": a})
    return in_maps


def kernel(v1s, v2s, cmaps, rid_to_vid):
    v1s = np.asarray(v1s)
    v2s = np.asarray(v2s)
    cmaps = np.asarray(cmaps)
    rid_to_vid = np.asarray(rid_to_vid)

    nc = _build()
    in_maps = _prep_inputs(v1s, v2s, rid_to_vid)
    res = run_bass_kernel_spmd(nc, in_maps, core_ids=list(range(NCORES)))

    # assemble [B, R, R] min squared distances (r = person1 region rows)
    md2 = np.empty((B, R, R), np.float32)
    for core in range(NCORES):
        b, h = divmod(core, 2)
        out = res.results[core]["s1out"]  # [128, NK*G], k = c*T + t
        # (p, c, t, g) -> v = t*128 + p, s = c*G + g
        per_v = out.reshape(128, NWC, T, G).transpose(2, 0, 1, 3).reshape(V, R)
        # segmented min over each region's 96 rows
        md2[b, RH * h : RH * (h + 1), :] = per_v.reshape(RH, VR, R).min(axis=1)

    md = np.sqrt(np.maximum(md2, 0.0))
    m = cmaps.astype(np.float32)
    return ((md * m).sum(axis=(1, 2)) / m.sum(axis=(1, 2))).astype(np.float32)
